# revision 1
# baseline (speedup 1.0000x reference)
"""GCN (DGL GraphConv x3 + residual + FC) on 8 Trainium2 NeuronCores.

Sharding: nodes are range-partitioned across the 8 cores (6250 nodes each).
Each core owns the edges whose dst falls in its shard.  Per layer, every core
computes the dense transform for its node shard (feat-major activations so no
transposes are ever needed), all-gathers the resulting 50000x128 bf16 message
table, gathers its edges' source rows with dma_gather (edge-major [128e,128f]
tiles), and segment-sums them into PSUM via one-hot matmuls (edges are
pre-sorted by dst on the host, so each 128-dst block accumulates a handful of
edge tiles).  Degree scalings fold in at node granularity:
  out[d] = r_in[d] * sum_e  (x W)[src_e] * r_out[src_e]   (+ bias, relu)
Key simplification: the reference computes relu(gconv(x1,W2)) twice (branch
and main are identical), so only 3 graph convs are needed, and the final
relu(x3+x2) is the identity on already-relu'd tensors (x3,x2 >= 0).

dma_gather indices are int16, so the table is split into two 25000-row halves;
each core keeps two dst-sorted edge lists (src<25000 / src>=25000) padded
per (block, half) to a common tile count across cores so all 8 cores run the
same program (SPMD) with different data.
"""
import sys

sys.path.insert(0, "/opt/trn_rl_repo")

import numpy as np
import ml_dtypes

from concourse import bacc, mybir, tile
from concourse.bass_utils import run_bass_kernel_spmd

BF16 = ml_dtypes.bfloat16
F32 = mybir.dt.float32
BF = mybir.dt.bfloat16
I16 = mybir.dt.int16

N_NODES = 50000
N_EDGES = 600000
IN_F = 602
HID = 128
OUT_F = 41
NCORES = 8
SH = N_NODES // NCORES          # 6250 nodes per core
P = 128
NBLK = (SH + P - 1) // P        # 49 dst blocks (last has 106)
LASTM = SH - (NBLK - 1) * P     # 106
HALF = N_NODES // 2             # table half split for int16 indices
KCH = 5                         # ceil(602/128) k-chunks for layer 1
INF_PAD = KCH * P               # 640
CHUNK_TILES = 8                 # edge tiles per dma_gather call (1024 edges; >=2048 faults on HW)
ST_GROUP = 8                    # edge tiles per one-hot DVE op
PAD_SLOT = 1000.0               # one-hot compare value for pad slots (never matches)


# ----------------------------------------------------------------- host prep

def _wrap_idx16(idx):
    """dma_gather idx layout: elem i -> partition i%16, slot i//16; replicated
    to 128 partitions (8 gpsimd cores read identical copies)."""
    n = len(idx)
    w = np.asarray(idx, np.int16).reshape(n // 16, 16).T
    return np.tile(w, (8, 1))


def _preprocess(features, src, dst, W1, b1, W2, b2, W3, b3, Wfc, bfc):
    src = np.asarray(src).astype(np.int64)
    dst = np.asarray(dst).astype(np.int64)
    features = np.asarray(features, np.float32)

    core_of = dst // SH
    per_core = []  # (idxA, slotA, idxB, slotB) unpadded, per (block, half)
    nA = np.zeros((NCORES, NBLK), np.int64)
    nB = np.zeros((NCORES, NBLK), np.int64)
    for c in range(NCORES):
        sel = core_of == c
        s = src[sel]
        dl = dst[sel] - c * SH
        order = np.argsort(dl, kind="stable")
        s, dl = s[order], dl[order]
        blk = dl >> 7
        slot = dl & 127
        isA = s < HALF
        blocksA, blocksB = [], []
        for b in range(NBLK):
            inb = blk == b
            a = inb & isA
            bb = inb & ~isA
            blocksA.append((s[a], slot[a]))
            blocksB.append((s[bb] - HALF, slot[bb]))
            nA[c, b] = a.sum()
            nB[c, b] = bb.sum()
        per_core.append((blocksA, blocksB))

    # common tile counts per (block, half) across cores
    TA = np.maximum(1, np.ceil(nA.max(0) / P).astype(np.int64))
    TB = np.maximum(1, np.ceil(nB.max(0) / P).astype(np.int64))
    TA_tot, TB_tot = int(TA.sum()), int(TB.sum())

    def build_half(blocks, T):
        idx = np.zeros(int(T.sum()) * P, np.int16)
        slot = np.full(int(T.sum()) * P, PAD_SLOT, np.float32)
        off = 0
        for b in range(NBLK):
            i, sl = blocks[b]
            n = len(i)
            idx[off:off + n] = i
            slot[off:off + n] = sl
            off += int(T[b]) * P
        return idx, slot

    in_maps = []
    deg_out_full = np.bincount(src, minlength=N_NODES).astype(np.float32)
    cum_out = np.concatenate([[0.0], np.cumsum(deg_out_full)]).astype(np.float32)

    featT = np.zeros((INF_PAD, N_NODES), np.float32)
    featT[:IN_F] = features.T
    W1p = np.zeros((INF_PAD, HID), np.float32)
    W1p[:IN_F] = W1

    for c in range(NCORES):
        blocksA, blocksB = per_core[c]
        idxA, slotA = build_half(blocksA, TA)
        idxB, slotB = build_half(blocksB, TB)
        slotAB = np.concatenate([slotA, slotB]).reshape(TA_tot + TB_tot, P).T

        # in-degree bounds (dst-sorted cumulative positions), this core's shard
        deg_in = np.bincount(dst[core_of == c] - c * SH, minlength=SH)
        cum_in = np.concatenate([[0], np.cumsum(deg_in)]).astype(np.float32)

        # out-degree bounds for this shard, node-partition-wrapped [128, NBLK]
        lo = np.zeros(NBLK * P, np.float32)
        hi = np.zeros(NBLK * P, np.float32)
        lo[:SH] = cum_out[c * SH: (c + 1) * SH]
        hi[:SH] = cum_out[c * SH + 1: (c + 1) * SH + 1]
        b_out_lo = lo.reshape(NBLK, P).T.copy()
        b_out_hi = hi.reshape(NBLK, P).T.copy()

        in_maps.append({
            "featT": featT[:, c * SH:(c + 1) * SH].reshape(KCH, P, SH).astype(BF16),
            "W1c": W1p.reshape(KCH, P, HID).transpose(1, 0, 2).astype(BF16).copy(),
            "W2c": W2.astype(BF16), "W3c": W3.astype(BF16),
            "Wfcc": Wfc.astype(BF16),
            "b1c": b1.reshape(HID, 1).astype(np.float32),
            "b2c": b2.reshape(HID, 1).astype(np.float32),
            "b3c": b3.reshape(HID, 1).astype(np.float32),
            "bfcc": bfc.reshape(1, OUT_F).astype(BF16),
            "iota": np.tile(np.arange(P, dtype=np.float32).astype(BF16), (P, 1)),
            "ones_f": np.ones((1, P), np.float32),
            "ones_b": np.ones((1, P), BF16),
            "idxA": _wrap_idx16(idxA), "idxB": _wrap_idx16(idxB),
            "slotAB": slotAB.astype(BF16),
            "b_in_lo": cum_in[:SH].reshape(1, SH),
            "b_in_hi": cum_in[1:SH + 1].reshape(1, SH),
            "b_out_lo": b_out_lo, "b_out_hi": b_out_hi,
        })

    sched = {"TA": TA.tolist(), "TB": TB.tolist(),
             "TA_tot": TA_tot, "TB_tot": TB_tot}
    return in_maps, sched


# ------------------------------------------------------------- device program

def _build(sched):
    TA, TB = sched["TA"], sched["TB"]
    TA_tot, TB_tot = sched["TA_tot"], sched["TB_tot"]
    EA, EB = TA_tot * P, TB_tot * P

    nc = bacc.Bacc("TRN2", target_bir_lowering=False, debug=False,
                   num_devices=NCORES, num_swdge_queues=4)

    def din(name, shape, dt):
        return nc.dram_tensor(name, shape, dt, kind="ExternalInput")

    h = {
        "featT": din("featT", [KCH, P, SH], BF),
        "W1c": din("W1c", [P, KCH, HID], BF),
        "W2c": din("W2c", [HID, HID], BF),
        "W3c": din("W3c", [HID, HID], BF),
        "Wfcc": din("Wfcc", [HID, OUT_F], BF),
        "b1c": din("b1c", [HID, 1], F32),
        "b2c": din("b2c", [HID, 1], F32),
        "b3c": din("b3c", [HID, 1], F32),
        "bfcc": din("bfcc", [1, OUT_F], BF),
        "iota": din("iota", [P, P], BF),
        "ones_f": din("ones_f", [1, P], F32),
        "ones_b": din("ones_b", [1, P], BF),
        "idxA": din("idxA", [P, EA // 16], I16),
        "idxB": din("idxB", [P, EB // 16], I16),
        "slotAB": din("slotAB", [P, TA_tot + TB_tot], BF),
        "b_in_lo": din("b_in_lo", [1, SH], F32),
        "b_in_hi": din("b_in_hi", [1, SH], F32),
        "b_out_lo": din("b_out_lo", [P, NBLK], F32),
        "b_out_hi": din("b_out_hi", [P, NBLK], F32),
    }
    out_fc = nc.dram_tensor("out_fc", [SH, OUT_F], F32, kind="ExternalOutput")
    Tshard = nc.dram_tensor("Tshard", [SH, HID], BF)
    Tfull = nc.dram_tensor("Tfull", [N_NODES, HID], BF, addr_space="Shared")

    with tile.TileContext(nc) as tc, \
         tc.tile_pool(name="const", bufs=1) as cp, \
         tc.tile_pool(name="state", bufs=1) as statep, \
         tc.tile_pool(name="feat", bufs=2 * KCH) as featp, \
         tc.tile_pool(name="msgA", bufs=10) as msgAp, \
         tc.tile_pool(name="msgB", bufs=10) as msgBp, \
         tc.tile_pool(name="sTA", bufs=6) as sTAp, \
         tc.tile_pool(name="sTB", bufs=6) as sTBp, \
         tc.tile_pool(name="stage", bufs=3) as stagep, \
         tc.tile_pool(name="ps_agg", bufs=4, space="PSUM") as ps_agg, \
         tc.tile_pool(name="ps_dense", bufs=2, space="PSUM") as ps_dense, \
         tc.tile_pool(name="ps_misc", bufs=1, space="PSUM") as ps_misc, \
         tc.tile_pool(name="ps_fc", bufs=1, space="PSUM") as ps_fc:

        def load(name, shape, dt):
            t = cp.tile(shape, dt, tag=name)
            nc.sync.dma_start(out=t[:], in_=h[name][:])
            return t

        W1s = load("W1c", [P, KCH, HID], BF)
        W2s = load("W2c", [HID, HID], BF)
        W3s = load("W3c", [HID, HID], BF)
        Wfcs = load("Wfcc", [HID, OUT_F], BF)
        b1s = load("b1c", [HID, 1], F32)
        b2s = load("b2c", [HID, 1], F32)
        b3s = load("b3c", [HID, 1], F32)
        bfcs = load("bfcc", [1, OUT_F], BF)
        iota = load("iota", [P, P], BF)
        ones_f = load("ones_f", [1, P], F32)
        ones_b = load("ones_b", [1, P], BF)
        idxA = load("idxA", [P, EA // 16], I16)
        idxB = load("idxB", [P, EB // 16], I16)
        slotAB = load("slotAB", [P, TA_tot + TB_tot], BF)
        bol = load("b_out_lo", [P, NBLK], F32)
        boh = load("b_out_hi", [P, NBLK], F32)

        # degrees -> r = 1/sqrt(max(deg,1))
        r_out = cp.tile([P, NBLK], F32, tag="r_out")
        nc.vector.tensor_sub(out=r_out[:], in0=boh[:], in1=bol[:])
        nc.vector.tensor_scalar_max(out=r_out[:], in0=r_out[:], scalar1=1.0)
        nc.scalar.activation(r_out[:], r_out[:], mybir.ActivationFunctionType.Sqrt)
        nc.vector.reciprocal(out=r_out[:], in_=r_out[:])

        # r_in = 1/sqrt(max(deg_in,1)), computed in 512-col chunks and
        # broadcast to [128, SH] via K=1 matmuls
        r_in_b = statep.tile([P, SH], F32, tag="r_in_b")
        for j in range(0, SH, 512):
            w = min(512, SH - j)
            blo = featp.tile([1, 512], F32, tag="blo")
            bhi = featp.tile([1, 512], F32, tag="bhi")
            nc.sync.dma_start(out=blo[:, :w], in_=h["b_in_lo"][:, j:j + w])
            nc.sync.dma_start(out=bhi[:, :w], in_=h["b_in_hi"][:, j:j + w])
            nc.vector.tensor_sub(out=blo[:, :w], in0=bhi[:, :w], in1=blo[:, :w])
            nc.vector.tensor_scalar_max(out=blo[:, :w], in0=blo[:, :w], scalar1=1.0)
            nc.scalar.activation(blo[:, :w], blo[:, :w],
                                 mybir.ActivationFunctionType.Sqrt)
            nc.vector.reciprocal_approx_fast(out=blo[:, :w], in_=blo[:, :w])
            pm = ps_misc.tile([P, 512], F32, space="PSUM")
            nc.tensor.matmul(pm[:, :w], lhsT=ones_f[:], rhs=blo[:, :w],
                             start=True, stop=True)
            nc.vector.tensor_copy(out=r_in_b[:, j:j + w], in_=pm[:, :w])

        agg = statep.tile([P, SH], F32, tag="agg")
        x1 = statep.tile([P, SH], BF, tag="x1")
        x2 = statep.tile([P, SH], BF, tag="x2")

        # ---- T1 = r_out * (features @ W1), feature tiles streamed from DRAM
        for nb in range(NBLK):
            m = P if nb < NBLK - 1 else LASTM
            ps = ps_dense.tile([P, HID], F32, space="PSUM")
            for k in range(KCH):
                ft = featp.tile([P, P], BF, tag="ft")
                nc.sync.dma_start(out=ft[:, :m], in_=h["featT"][k][:, nb * P:nb * P + m])
                nc.tensor.matmul(ps[:m, :],
                                 lhsT=ft[:, :m],
                                 rhs=W1s[:, k, :],
                                 start=(k == 0), stop=(k == KCH - 1))
            st = stagep.tile([P, HID], BF, tag="st")
            nc.vector.tensor_scalar_mul(out=st[:m, :], in0=ps[:m, :],
                                        scalar1=r_out[:m, nb:nb + 1])
            nc.sync.dma_start(out=Tshard[nb * P:nb * P + m, :], in_=st[:m, :])
        nc.gpsimd.collective_compute(
            "AllGather", mybir.AluOpType.bypass,
            replica_groups=[list(range(NCORES))],
            ins=[Tshard[:].opt()], outs=[Tfull[:].opt()])

        # ---- aggregation machinery
        def emit_gather(tot_tiles, idx_t, base_ap, pool, k, q):
            nt = min(CHUNK_TILES, tot_tiles - k * CHUNK_TILES)
            mt = pool.tile([P, nt, HID], BF, tag="msg")
            nidx = nt * P
            c0 = k * CHUNK_TILES * P // 16
            nc.gpsimd.dma_gather(
                out_ap=mt[:], in_ap=base_ap, idxs_ap=idx_t[:, c0:c0 + nidx // 16],
                num_idxs=nidx, num_idxs_reg=nidx, elem_size=HID,
                queue_num=q % 4)
            return mt

        def gathers_interleaved():
            """Emit A/B gather calls alternating (matches per-block A-then-B
            consumption order) round-robin across the 8 SWDGE queues."""
            ncA = (TA_tot + CHUNK_TILES - 1) // CHUNK_TILES
            ncB = (TB_tot + CHUNK_TILES - 1) // CHUNK_TILES
            mA, mB = [], []
            q = 0
            for k in range(max(ncA, ncB)):
                if k < ncA:
                    mA.append(emit_gather(TA_tot, idxA, Tfull[0:HALF, :], msgAp, k, q))
                    q += 1
                if k < ncB:
                    mB.append(emit_gather(TB_tot, idxB, Tfull[HALF:N_NODES, :], msgBp, k, q))
                    q += 1
            return mA, mB

        def st_group(pool, slot_off, g, tot_tiles):
            nt = min(ST_GROUP, tot_tiles - g * ST_GROUP)
            t = pool.tile([P, ST_GROUP, P], BF, tag="sT")
            sl = slotAB[:, slot_off + g * ST_GROUP: slot_off + g * ST_GROUP + nt]
            nc.vector.tensor_tensor(
                out=t[:, :nt, :],
                in0=sl.unsqueeze(2).to_broadcast([P, nt, P]),
                in1=iota[:].unsqueeze(1).to_broadcast([P, nt, P]),
                op=mybir.AluOpType.is_equal)
            return t

        def aggregate(bias, xout):
            mA, mB = gathers_interleaved()
            sA, sB = {}, {}
            tA = tB = 0
            for b in range(NBLK):
                ps = ps_agg.tile([P, P], F32, space="PSUM")
                tot = TA[b] + TB[b]
                i = 0
                for (cnt, cur, msgs, sTs, pool, soff, ttot) in (
                        (TA[b], tA, mA, sA, sTAp, 0, TA_tot),
                        (TB[b], tB, mB, sB, sTBp, TA_tot, TB_tot)):
                    for t in range(cur, cur + cnt):
                        g = t // ST_GROUP
                        if g not in sTs:
                            sTs[g] = st_group(pool, soff, g, ttot)
                        nc.tensor.matmul(
                            ps[:],
                            lhsT=msgs[t // CHUNK_TILES][:, t % CHUNK_TILES, :],
                            rhs=sTs[g][:, t % ST_GROUP, :],
                            start=(i == 0), stop=(i == tot - 1))
                        i += 1
                tA += TA[b]
                tB += TB[b]
                m = P if b < NBLK - 1 else LASTM
                nc.vector.tensor_mul(out=agg[:, b * P:b * P + m], in0=ps[:, :m],
                                     in1=r_in_b[:, b * P:b * P + m])
            nc.scalar.activation(xout[:], agg[:],
                                 mybir.ActivationFunctionType.Relu, bias=bias[:])

        def dense_to_table(xin, Wt):
            for nb in range(NBLK):
                m = P if nb < NBLK - 1 else LASTM
                ps = ps_dense.tile([P, HID], F32, space="PSUM")
                nc.tensor.matmul(ps[:m, :], lhsT=xin[:, nb * P:nb * P + m],
                                 rhs=Wt[:], start=True, stop=True)
                st = stagep.tile([P, HID], BF, tag="st")
                nc.vector.tensor_scalar_mul(out=st[:m, :], in0=ps[:m, :],
                                            scalar1=r_out[:m, nb:nb + 1])
                nc.sync.dma_start(out=Tshard[nb * P:nb * P + m, :], in_=st[:m, :])
            nc.gpsimd.collective_compute(
                "AllGather", mybir.AluOpType.bypass,
                replica_groups=[list(range(NCORES))],
                ins=[Tshard[:].opt()], outs=[Tfull[:].opt()])

        aggregate(b1s, x1)          # x1 = relu(gconv(features, W1))
        dense_to_table(x1, W2s)     # T2
        aggregate(b2s, x2)          # x2 = relu(gconv(x1, W2))
        dense_to_table(x2, W3s)     # T3
        x3 = statep.tile([P, SH], BF, tag="x1")   # reuse x1 slot
        aggregate(b3s, x3)          # x3 = relu(gconv(x2, W3))

        # x4 = relu(x3 + x2) == x3 + x2 (both already >= 0); in-place into x2
        x4 = x2
        nc.vector.tensor_add(out=x4[:], in0=x3[:], in1=x2[:])

        # out = x4 @ Wfc + bfc
        for nb in range(NBLK):
            m = P if nb < NBLK - 1 else LASTM
            ps = ps_fc.tile([P, OUT_F], F32, space="PSUM")
            nc.tensor.matmul(ps[:m, :], lhsT=x4[:, nb * P:nb * P + m],
                             rhs=Wfcs[:], start=True, stop=False)
            nc.tensor.matmul(ps[:m, :], lhsT=ones_b[:, :m], rhs=bfcs[:],
                             start=False, stop=True)
            st = stagep.tile([P, OUT_F], F32, tag="stf")
            nc.vector.tensor_copy(out=st[:m, :], in_=ps[:m, :])
            nc.sync.dma_start(out=out_fc[nb * P:nb * P + m, :], in_=st[:m, :])

    nc.compile()
    return nc


_CACHED = None


def kernel(**inputs):
    global _CACHED
    in_maps, sched = _preprocess(**inputs)
    if _CACHED is None or _CACHED[1] != sched:
        _CACHED = (_build(sched), sched)
    nc = _CACHED[0]
    res = run_bass_kernel_spmd(nc, in_maps, list(range(NCORES)))
    return np.concatenate(
        [np.asarray(res.results[c]["out_fc"], np.float32) for c in range(NCORES)], 0)



# revision 8
# speedup vs baseline: 1.4207x; 1.4207x over previous
"""GCN (DGL GraphConv x3 + residual + FC) on 8 Trainium2 NeuronCores.

Sharding: nodes are range-partitioned across the 8 cores (6250 nodes each).
Each core owns the edges whose dst falls in its shard.  Per layer, every core
computes the dense transform for its node shard (feat-major activations so no
transposes are ever needed), all-gathers the resulting 50000x128 bf16 message
table, gathers its edges' source rows with dma_gather (edge-major [128e,128f]
tiles), and segment-sums them into PSUM via one-hot matmuls (edges are
pre-sorted by dst on the host, so each 128-dst block accumulates a handful of
edge tiles).  Degree scalings fold in at node granularity:
  out[d] = r_in[d] * sum_e  (x W)[src_e] * r_out[src_e]   (+ bias, relu)
Key simplification: the reference computes relu(gconv(x1,W2)) twice (branch
and main are identical), so only 3 graph convs are needed, and the final
relu(x3+x2) is the identity on already-relu'd tensors (x3,x2 >= 0).

v3 perf changes vs baseline:
  - featT streamed in 8-block slabs (one 1.3MB DMA each) instead of 245
    32KB tile DMAs (removes ~200us of startup serialization).
  - per-block tail fusion: scale/relu plus the NEXT layer's dense matmul
    (or the final residual+FC) run per 128-dst block inside the aggregation
    loop, so each layer's serial tail shrinks from ~60us to ~2us.
  - edges sorted by src within each (block, half): gather descriptors hit
    ascending HBM addresses (row locality for the SDMA random reads).
  - pad gather indices spread over the table instead of all hitting row 0.
  - per-layer Tshard/Tfull DRAM buffers (no WAR hazards across layers).

dma_gather indices are int16, so the table is split into two 25000-row halves;
each core keeps two dst-sorted edge lists (src<25000 / src>=25000) padded
per (block, half) to a common tile count across cores so all 8 cores run the
same program (SPMD) with different data.
"""
import sys

sys.path.insert(0, "/opt/trn_rl_repo")

import numpy as np
import ml_dtypes

from concourse import bacc, mybir, tile
from concourse.bass_utils import run_bass_kernel_spmd

BF16 = ml_dtypes.bfloat16
F32 = mybir.dt.float32
BF = mybir.dt.bfloat16
I16 = mybir.dt.int16

N_NODES = 50000
N_EDGES = 600000
IN_F = 602
HID = 128
OUT_F = 41
NCORES = 8
SH = N_NODES // NCORES          # 6250 nodes per core
P = 128
NBLK = (SH + P - 1) // P        # 49 dst blocks (last has 106)
LASTM = SH - (NBLK - 1) * P     # 106
HALF = N_NODES // 2             # table half split for int16 indices
KCH = 5                         # ceil(602/128) k-chunks for layer 1
INF_PAD = KCH * P               # 640
CHUNK_TILES = 8                 # edge tiles per dma_gather call (1024 edges; >=2048 faults on HW)
ST_GROUP = 8                    # edge tiles per one-hot DVE op
PAD_SLOT = 1000.0               # one-hot compare value for pad slots (never matches)
SLABW = 1024                    # featT slab width (8 dst blocks)


# ----------------------------------------------------------------- host prep

def _wrap_idx16(idx):
    """dma_gather idx layout: elem i -> partition i%16, slot i//16; replicated
    to 128 partitions (8 gpsimd cores read identical copies)."""
    n = len(idx)
    w = np.asarray(idx, np.int16).reshape(n // 16, 16).T
    return np.tile(w, (8, 1))


def _preprocess(features, src, dst, W1, b1, W2, b2, W3, b3, Wfc, bfc):
    src = np.asarray(src).astype(np.int64)
    dst = np.asarray(dst).astype(np.int64)
    features = np.asarray(features, np.float32)

    core_of = dst // SH
    per_core = []  # (idxA, slotA, idxB, slotB) unpadded, per (block, half)
    nA = np.zeros((NCORES, NBLK), np.int64)
    nB = np.zeros((NCORES, NBLK), np.int64)
    for c in range(NCORES):
        sel = core_of == c
        s = src[sel]
        dl = dst[sel] - c * SH
        order = np.argsort(dl, kind="stable")
        s, dl = s[order], dl[order]
        blk = dl >> 7
        slot = dl & 127
        isA = s < HALF
        blocksA, blocksB = [], []
        for b in range(NBLK):
            inb = blk == b
            for half_mask, blocks, base in ((inb & isA, blocksA, 0),
                                            (inb & ~isA, blocksB, HALF)):
                bs, bsl = s[half_mask] - base, slot[half_mask]
                so = np.argsort(bs, kind="stable")  # src-sorted within block
                blocks.append((bs[so], bsl[so]))
            nA[c, b] = (inb & isA).sum()
            nB[c, b] = (inb & ~isA).sum()
        per_core.append((blocksA, blocksB))

    # common tile counts per (block, half) across cores
    TA = np.maximum(1, np.ceil(nA.max(0) / P).astype(np.int64))
    TB = np.maximum(1, np.ceil(nB.max(0) / P).astype(np.int64))
    TA_tot, TB_tot = int(TA.sum()), int(TB.sum())

    def build_half(blocks, T):
        n_tot = int(T.sum()) * P
        # spread pad indices over the half-table so padded gathers don't all
        # hammer row 0 of HBM
        idx = (np.arange(n_tot, dtype=np.int64) * 197 % HALF).astype(np.int16)
        slot = np.full(n_tot, PAD_SLOT, np.float32)
        off = 0
        for b in range(NBLK):
            i, sl = blocks[b]
            n = len(i)
            idx[off:off + n] = i
            slot[off:off + n] = sl
            off += int(T[b]) * P
        return idx, slot

    in_maps = []
    deg_out_full = np.bincount(src, minlength=N_NODES).astype(np.float32)
    cum_out = np.concatenate([[0.0], np.cumsum(deg_out_full)]).astype(np.float32)

    featT = np.zeros((INF_PAD, N_NODES), np.float32)
    featT[:IN_F] = features.T
    W1p = np.zeros((INF_PAD, HID), np.float32)
    W1p[:IN_F] = W1

    for c in range(NCORES):
        blocksA, blocksB = per_core[c]
        idxA, slotA = build_half(blocksA, TA)
        idxB, slotB = build_half(blocksB, TB)
        slotAB = np.concatenate([slotA, slotB]).reshape(TA_tot + TB_tot, P).T

        # in-degree bounds (dst-sorted cumulative positions), this core's shard
        deg_in = np.bincount(dst[core_of == c] - c * SH, minlength=SH)
        cum_in = np.concatenate([[0], np.cumsum(deg_in)]).astype(np.float32)

        # out-degree bounds for this shard, node-partition-wrapped [128, NBLK]
        lo = np.zeros(NBLK * P, np.float32)
        hi = np.zeros(NBLK * P, np.float32)
        lo[:SH] = cum_out[c * SH: (c + 1) * SH]
        hi[:SH] = cum_out[c * SH + 1: (c + 1) * SH + 1]
        b_out_lo = lo.reshape(NBLK, P).T.copy()
        b_out_hi = hi.reshape(NBLK, P).T.copy()

        # featT shard as [P, KCH, SH] so one 3D-AP DMA loads a [128,KCH,slab]
        featTc = featT[:, c * SH:(c + 1) * SH].reshape(KCH, P, SH)
        in_maps.append({
            "featT": featTc.transpose(1, 0, 2).astype(BF16).copy(),
            "W1c": W1p.reshape(KCH, P, HID).transpose(1, 0, 2).astype(BF16).copy(),
            "W2c": W2.astype(BF16), "W3c": W3.astype(BF16),
            "Wfcc": Wfc.astype(BF16),
            "b1c": b1.reshape(HID, 1).astype(np.float32),
            "b2c": b2.reshape(HID, 1).astype(np.float32),
            "b3c": b3.reshape(HID, 1).astype(np.float32),
            "bfcc": bfc.reshape(1, OUT_F).astype(BF16),
            "iota": np.tile(np.arange(P, dtype=np.float32).astype(BF16), (P, 1)),
            "ones_f": np.ones((1, P), np.float32),
            "ones_b": np.ones((1, P), BF16),
            "idxA": _wrap_idx16(idxA), "idxB": _wrap_idx16(idxB),
            "slotAB": slotAB.astype(BF16),
            "b_in_lo": cum_in[:SH].reshape(1, SH),
            "b_in_hi": cum_in[1:SH + 1].reshape(1, SH),
            "b_out_lo": b_out_lo, "b_out_hi": b_out_hi,
        })

    sched = {"TA": TA.tolist(), "TB": TB.tolist(),
             "TA_tot": TA_tot, "TB_tot": TB_tot}
    return in_maps, sched


# ------------------------------------------------------------- device program

def _build(sched):
    TA, TB = sched["TA"], sched["TB"]
    TA_tot, TB_tot = sched["TA_tot"], sched["TB_tot"]
    EA, EB = TA_tot * P, TB_tot * P

    nc = bacc.Bacc("TRN2", target_bir_lowering=False, debug=False,
                   num_devices=NCORES, num_swdge_queues=4)

    def din(name, shape, dt):
        return nc.dram_tensor(name, shape, dt, kind="ExternalInput")

    h = {
        "featT": din("featT", [P, KCH, SH], BF),
        "W1c": din("W1c", [P, KCH, HID], BF),
        "W2c": din("W2c", [HID, HID], BF),
        "W3c": din("W3c", [HID, HID], BF),
        "Wfcc": din("Wfcc", [HID, OUT_F], BF),
        "b1c": din("b1c", [HID, 1], F32),
        "b2c": din("b2c", [HID, 1], F32),
        "b3c": din("b3c", [HID, 1], F32),
        "bfcc": din("bfcc", [1, OUT_F], BF),
        "iota": din("iota", [P, P], BF),
        "ones_f": din("ones_f", [1, P], F32),
        "ones_b": din("ones_b", [1, P], BF),
        "idxA": din("idxA", [P, EA // 16], I16),
        "idxB": din("idxB", [P, EB // 16], I16),
        "slotAB": din("slotAB", [P, TA_tot + TB_tot], BF),
        "b_in_lo": din("b_in_lo", [1, SH], F32),
        "b_in_hi": din("b_in_hi", [1, SH], F32),
        "b_out_lo": din("b_out_lo", [P, NBLK], F32),
        "b_out_hi": din("b_out_hi", [P, NBLK], F32),
    }
    out_fc = nc.dram_tensor("out_fc", [SH, OUT_F], F32, kind="ExternalOutput")
    Tshard = [nc.dram_tensor(f"Tshard{i}", [SH, HID], BF) for i in range(3)]
    Tfull = [nc.dram_tensor(f"Tfull{i}", [N_NODES, HID], BF, addr_space="Shared")
             for i in range(3)]

    with tile.TileContext(nc) as tc, \
         tc.tile_pool(name="const", bufs=1) as cp, \
         tc.tile_pool(name="state", bufs=1) as statep, \
         tc.tile_pool(name="feat", bufs=2) as featp, \
         tc.tile_pool(name="msgA", bufs=10) as msgAp, \
         tc.tile_pool(name="msgB", bufs=10) as msgBp, \
         tc.tile_pool(name="sTA", bufs=6) as sTAp, \
         tc.tile_pool(name="sTB", bufs=6) as sTBp, \
         tc.tile_pool(name="stage", bufs=8) as stagep, \
         tc.tile_pool(name="ps_agg", bufs=4, space="PSUM") as ps_agg, \
         tc.tile_pool(name="ps_dense", bufs=2, space="PSUM") as ps_dense, \
         tc.tile_pool(name="ps_misc", bufs=1, space="PSUM") as ps_misc, \
         tc.tile_pool(name="ps_fc", bufs=1, space="PSUM") as ps_fc:

        def load(name, shape, dt):
            t = cp.tile(shape, dt, tag=name)
            nc.sync.dma_start(out=t[:], in_=h[name][:])
            return t

        W1s = load("W1c", [P, KCH, HID], BF)
        W2s = load("W2c", [HID, HID], BF)
        W3s = load("W3c", [HID, HID], BF)
        Wfcs = load("Wfcc", [HID, OUT_F], BF)
        b1s = load("b1c", [HID, 1], F32)
        b2s = load("b2c", [HID, 1], F32)
        b3s = load("b3c", [HID, 1], F32)
        bfcs = load("bfcc", [1, OUT_F], BF)
        iota = load("iota", [P, P], BF)
        ones_f = load("ones_f", [1, P], F32)
        ones_b = load("ones_b", [1, P], BF)
        idxA = load("idxA", [P, EA // 16], I16)
        idxB = load("idxB", [P, EB // 16], I16)
        slotAB = load("slotAB", [P, TA_tot + TB_tot], BF)
        bol = load("b_out_lo", [P, NBLK], F32)
        boh = load("b_out_hi", [P, NBLK], F32)

        # degrees -> r = 1/sqrt(max(deg,1))
        r_out = cp.tile([P, NBLK], F32, tag="r_out")
        nc.vector.tensor_sub(out=r_out[:], in0=boh[:], in1=bol[:])
        nc.vector.tensor_scalar_max(out=r_out[:], in0=r_out[:], scalar1=1.0)
        nc.scalar.activation(r_out[:], r_out[:], mybir.ActivationFunctionType.Sqrt)
        nc.vector.reciprocal(out=r_out[:], in_=r_out[:])

        # r_in = 1/sqrt(max(deg_in,1)), computed in 512-col chunks and
        # broadcast to [128, SH] via K=1 matmuls
        r_in_b = statep.tile([P, SH], F32, tag="r_in_b")
        for j in range(0, SH, 512):
            w = min(512, SH - j)
            blo = stagep.tile([1, 512], F32, tag="blo")
            bhi = stagep.tile([1, 512], F32, tag="bhi")
            nc.sync.dma_start(out=blo[:, :w], in_=h["b_in_lo"][:, j:j + w])
            nc.sync.dma_start(out=bhi[:, :w], in_=h["b_in_hi"][:, j:j + w])
            nc.vector.tensor_sub(out=blo[:, :w], in0=bhi[:, :w], in1=blo[:, :w])
            nc.vector.tensor_scalar_max(out=blo[:, :w], in0=blo[:, :w], scalar1=1.0)
            nc.scalar.activation(blo[:, :w], blo[:, :w],
                                 mybir.ActivationFunctionType.Sqrt)
            nc.vector.reciprocal_approx_fast(out=blo[:, :w], in_=blo[:, :w])
            pm = ps_misc.tile([P, 512], F32, space="PSUM")
            nc.tensor.matmul(pm[:, :w], lhsT=ones_f[:], rhs=blo[:, :w],
                             start=True, stop=True)
            nc.vector.tensor_copy(out=r_in_b[:, j:j + w], in_=pm[:, :w])

        x1 = statep.tile([P, SH], BF, tag="x1")
        x2 = statep.tile([P, SH], BF, tag="x2")

        def dense_block(li, xin, Wt, nb, m):
            """Tshard[li][block nb] = r_out * (xin[:, block nb] @ Wt)."""
            ps = ps_dense.tile([P, HID], F32, space="PSUM")
            nc.tensor.matmul(ps[:m, :], lhsT=xin[:, nb * P:nb * P + m],
                             rhs=Wt[:], start=True, stop=True)
            st = stagep.tile([P, HID], BF, tag="st")
            nc.vector.tensor_scalar_mul(out=st[:m, :], in0=ps[:m, :],
                                        scalar1=r_out[:m, nb:nb + 1])
            nc.sync.dma_start(out=Tshard[li][nb * P:nb * P + m, :], in_=st[:m, :])

        def all_gather(li):
            nc.gpsimd.collective_compute(
                "AllGather", mybir.AluOpType.bypass,
                replica_groups=[list(range(NCORES))],
                ins=[Tshard[li][:].opt()], outs=[Tfull[li][:].opt()])

        # ---- T1 = r_out * (features @ W1), feature slabs streamed from DRAM
        for j0 in range(0, SH, SLABW):
            w = min(SLABW, SH - j0)
            ft = featp.tile([P, KCH, SLABW], BF, tag="ft")
            nc.sync.dma_start(out=ft[:, :, :w], in_=h["featT"][:, :, j0:j0 + w])
            for nb in range(j0 // P, (j0 + w + P - 1) // P):
                c0 = nb * P - j0
                m = min(P, w - c0)
                ps = ps_dense.tile([P, HID], F32, space="PSUM")
                for k in range(KCH):
                    nc.tensor.matmul(ps[:m, :],
                                     lhsT=ft[:, k, c0:c0 + m],
                                     rhs=W1s[:, k, :],
                                     start=(k == 0), stop=(k == KCH - 1))
                st = stagep.tile([P, HID], BF, tag="st")
                nc.vector.tensor_scalar_mul(out=st[:m, :], in0=ps[:m, :],
                                            scalar1=r_out[:m, nb:nb + 1])
                nc.sync.dma_start(out=Tshard[0][nb * P:nb * P + m, :], in_=st[:m, :])
        all_gather(0)

        # ---- aggregation machinery
        def emit_gather(li, tot_tiles, idx_t, base_ap, pool, k, q):
            nt = min(CHUNK_TILES, tot_tiles - k * CHUNK_TILES)
            mt = pool.tile([P, nt, HID], BF, tag="msg")
            nidx = nt * P
            c0 = k * CHUNK_TILES * P // 16
            nc.gpsimd.dma_gather(
                out_ap=mt[:], in_ap=base_ap, idxs_ap=idx_t[:, c0:c0 + nidx // 16],
                num_idxs=nidx, num_idxs_reg=nidx, elem_size=HID,
                queue_num=q % 4)
            return mt

        def gathers_interleaved(li):
            """Emit A/B gather calls alternating (matches per-block A-then-B
            consumption order) round-robin across the 4 SWDGE queues."""
            ncA = (TA_tot + CHUNK_TILES - 1) // CHUNK_TILES
            ncB = (TB_tot + CHUNK_TILES - 1) // CHUNK_TILES
            mA, mB = [], []
            q = 0
            for k in range(max(ncA, ncB)):
                if k < ncA:
                    mA.append(emit_gather(li, TA_tot, idxA,
                                          Tfull[li][0:HALF, :], msgAp, k, q))
                    q += 1
                if k < ncB:
                    mB.append(emit_gather(li, TB_tot, idxB,
                                          Tfull[li][HALF:N_NODES, :], msgBp, k, q))
                    q += 1
            return mA, mB

        def st_group(pool, slot_off, g, tot_tiles):
            nt = min(ST_GROUP, tot_tiles - g * ST_GROUP)
            t = pool.tile([P, ST_GROUP, P], BF, tag="sT")
            sl = slotAB[:, slot_off + g * ST_GROUP: slot_off + g * ST_GROUP + nt]
            nc.vector.tensor_tensor(
                out=t[:, :nt, :],
                in0=sl.unsqueeze(2).to_broadcast([P, nt, P]),
                in1=iota[:].unsqueeze(1).to_broadcast([P, nt, P]),
                op=mybir.AluOpType.is_equal)
            return t

        def aggregate(li, bias, xout, dense_W=None, fc=False):
            """Per dst block: one-hot matmul segment-sum, then fused tail:
            xout[:,blk] = relu(r_in * agg + bias); plus either the NEXT
            layer's dense+scale+store for this block (dense_W) or the final
            residual+FC+store (fc)."""
            mA, mB = gathers_interleaved(li)
            sA, sB = {}, {}
            tA = tB = 0
            for b in range(NBLK):
                ps = ps_agg.tile([P, P], F32, space="PSUM")
                tot = TA[b] + TB[b]
                i = 0
                for (cnt, cur, msgs, sTs, pool, soff, ttot) in (
                        (TA[b], tA, mA, sA, sTAp, 0, TA_tot),
                        (TB[b], tB, mB, sB, sTBp, TA_tot, TB_tot)):
                    for t in range(cur, cur + cnt):
                        g = t // ST_GROUP
                        if g not in sTs:
                            sTs[g] = st_group(pool, soff, g, ttot)
                        nc.tensor.matmul(
                            ps[:],
                            lhsT=msgs[t // CHUNK_TILES][:, t % CHUNK_TILES, :],
                            rhs=sTs[g][:, t % ST_GROUP, :],
                            start=(i == 0), stop=(i == tot - 1))
                        i += 1
                tA += TA[b]
                tB += TB[b]
                m = P if b < NBLK - 1 else LASTM
                bsl = slice(b * P, b * P + m)
                mulst = stagep.tile([P, P], BF, tag="mulst")
                nc.vector.tensor_mul(out=mulst[:, :m], in0=ps[:, :m],
                                     in1=r_in_b[:, bsl])
                nc.scalar.activation(xout[:, bsl], mulst[:, :m],
                                     mybir.ActivationFunctionType.Relu,
                                     bias=bias[:])
                if dense_W is not None:
                    dense_block(li + 1, xout, dense_W, b, m)
                if fc:
                    x4b = stagep.tile([P, P], BF, tag="x4b")
                    nc.vector.tensor_add(out=x4b[:, :m], in0=xout[:, bsl],
                                         in1=x2[:, bsl])
                    psf = ps_fc.tile([P, OUT_F], F32, space="PSUM")
                    nc.tensor.matmul(psf[:m, :], lhsT=x4b[:, :m],
                                     rhs=Wfcs[:], start=True, stop=False)
                    nc.tensor.matmul(psf[:m, :], lhsT=ones_b[:, :m], rhs=bfcs[:],
                                     start=False, stop=True)
                    stf = stagep.tile([P, OUT_F], F32, tag="stf")
                    nc.vector.tensor_copy(out=stf[:m, :], in_=psf[:m, :])
                    nc.sync.dma_start(out=out_fc[b * P:b * P + m, :],
                                      in_=stf[:m, :])
            if dense_W is not None:
                all_gather(li + 1)

        x3 = statep.tile([P, SH], BF, tag="x1")   # reuse x1 slot (x1 dead then)
        aggregate(0, b1s, x1, dense_W=W2s)   # x1 = relu(gconv(feat,W1)); T2; AG
        aggregate(1, b2s, x2, dense_W=W3s)   # x2 = relu(gconv(x1,W2));  T3; AG
        aggregate(2, b3s, x3, fc=True)       # x3 + residual + FC + store

    nc.compile()
    return nc


_CACHED = None


def kernel(**inputs):
    global _CACHED
    in_maps, sched = _preprocess(**inputs)
    if _CACHED is None or _CACHED[1] != sched:
        _CACHED = (_build(sched), sched)
    nc = _CACHED[0]
    res = run_bass_kernel_spmd(nc, in_maps, list(range(NCORES)))
    return np.concatenate(
        [np.asarray(res.results[c]["out_fc"], np.float32) for c in range(NCORES)], 0)


# revision 10
# speedup vs baseline: 1.4810x; 1.0424x over previous
"""GCN (DGL GraphConv x3 + residual + FC) on 8 Trainium2 NeuronCores.

Sharding: nodes are range-partitioned across the 8 cores (6250 nodes each).
Each core owns the edges whose dst falls in its shard.  Per layer, every core
computes the dense transform for its node shard (feat-major activations so no
transposes are ever needed), all-gathers the resulting 50000x128 bf16 message
table, gathers its edges' source rows with dma_gather (edge-major [128e,128f]
tiles), and segment-sums them into PSUM via one-hot matmuls (edges are
pre-sorted by dst on the host, so each 128-dst block accumulates a handful of
edge tiles).  Degree scalings fold in at node granularity:
  out[d] = r_in[d] * sum_e  (x W)[src_e] * r_out[src_e]   (+ bias, relu)
Key simplification: the reference computes relu(gconv(x1,W2)) twice (branch
and main are identical), so only 3 graph convs are needed, and the final
relu(x3+x2) is the identity on already-relu'd tensors (x3,x2 >= 0).

Perf structure (v5):
  - featT streamed in 8-block slabs (one 1.3MB DMA each).
  - per-block tail fusion: scale/relu plus the NEXT layer's dense matmul
    (or the final residual+FC) run per 128-dst block inside the aggregation
    loop, so each layer's serial tail is ~2us.
  - split AllGather: the message table is exchanged in two halves (each
    core's shard rows [0,3200) and [3200,6250)); the A-half collective is
    issued as soon as dst blocks 0..24 of the fused dense are stored,
    i.e. it overlaps the previous layer's gather drain, and the B-half
    collective overlaps the next layer's A-half gathers.
  - edges sorted by src within each (block, half): gather descriptors hit
    ascending HBM addresses (halves the SDMA per-descriptor cost).
  - r_in / r_out (degree rsqrt) precomputed on host like the rest of the
    graph indexing; r_in shipped pre-broadcast as [128, SH].
  - pad gather indices spread over the table instead of all hitting row 0.
  - per-layer Tshard/Tfull DRAM buffers (no WAR hazards across layers).

dma_gather indices are int16; the shard-half split also keeps every gather
index under 25600, comfortably inside int16 range.  Each core keeps two
dst-sorted edge lists (per half) padded per (block, half) to a common tile
count across cores so all 8 cores run the same program (SPMD).
"""
import sys

sys.path.insert(0, "/opt/trn_rl_repo")

import numpy as np
import ml_dtypes

from concourse import bacc, mybir, tile
from concourse.bass_utils import run_bass_kernel_spmd

BF16 = ml_dtypes.bfloat16
F32 = mybir.dt.float32
BF = mybir.dt.bfloat16
I16 = mybir.dt.int16

N_NODES = 50000
N_EDGES = 600000
IN_F = 602
HID = 128
OUT_F = 41
NCORES = 8
SH = N_NODES // NCORES          # 6250 nodes per core
P = 128
NBLK = (SH + P - 1) // P        # 49 dst blocks (last has 106)
LASTM = SH - (NBLK - 1) * P     # 106
ASH = 3200                      # A-half rows per core shard (blocks 0..24)
BSH = SH - ASH                  # 3050 B-half rows (blocks 25..48)
ABLK = ASH // P                 # 25
NA = NCORES * ASH               # 25600-row A table
NB = NCORES * BSH               # 24400-row B table
KCH = 5                         # ceil(602/128) k-chunks for layer 1
INF_PAD = KCH * P               # 640
CHUNK_TILES = 8                 # edge tiles per dma_gather call (1024 edges; >=2048 faults on HW)
ST_GROUP = 16                   # edge tiles per one-hot DVE op
PAD_SLOT = 1000.0               # one-hot compare value for pad slots (never matches)
SLABW = 1024                    # featT slab width (8 dst blocks)


# ----------------------------------------------------------------- host prep

def _wrap_idx16(idx):
    """dma_gather idx layout: elem i -> partition i%16, slot i//16; replicated
    to 128 partitions (8 gpsimd cores read identical copies)."""
    n = len(idx)
    w = np.asarray(idx, np.int16).reshape(n // 16, 16).T
    return np.tile(w, (8, 1))


def _preprocess(features, src, dst, W1, b1, W2, b2, W3, b3, Wfc, bfc):
    src = np.asarray(src).astype(np.int64)
    dst = np.asarray(dst).astype(np.int64)
    features = np.asarray(features, np.float32)

    # src relabel into the two shard-half tables
    s_core = src // SH
    s_row = src % SH
    src_isA = s_row < ASH
    src_idxA = s_core * ASH + s_row              # valid where src_isA
    src_idxB = s_core * BSH + (s_row - ASH)      # valid where ~src_isA

    core_of = dst // SH
    per_core = []  # (blocksA, blocksB) lists of (idx, slot) per dst block
    nA = np.zeros((NCORES, NBLK), np.int64)
    nB = np.zeros((NCORES, NBLK), np.int64)
    for c in range(NCORES):
        sel = core_of == c
        ia, ib = src_idxA[sel], src_idxB[sel]
        isA = src_isA[sel]
        dl = dst[sel] - c * SH
        order = np.argsort(dl, kind="stable")
        ia, ib, isA, dl = ia[order], ib[order], isA[order], dl[order]
        blk = dl >> 7
        slot = dl & 127
        blocksA, blocksB = [], []
        for b in range(NBLK):
            inb = blk == b
            for mask, idxs, blocks in ((inb & isA, ia, blocksA),
                                       (inb & ~isA, ib, blocksB)):
                bi, bsl = idxs[mask], slot[mask]
                so = np.argsort(bi, kind="stable")  # src-sorted within block
                blocks.append((bi[so], bsl[so]))
            nA[c, b] = (inb & isA).sum()
            nB[c, b] = (inb & ~isA).sum()
        per_core.append((blocksA, blocksB))

    # common tile counts per (block, half) across cores
    TA = np.maximum(1, np.ceil(nA.max(0) / P).astype(np.int64))
    TB = np.maximum(1, np.ceil(nB.max(0) / P).astype(np.int64))
    TA_tot, TB_tot = int(TA.sum()), int(TB.sum())

    def build_half(blocks, T, tbl_rows):
        n_tot = int(T.sum()) * P
        # spread pad indices over the table so padded gathers don't all
        # hammer row 0 of HBM
        idx = (np.arange(n_tot, dtype=np.int64) * 197 % tbl_rows).astype(np.int16)
        slot = np.full(n_tot, PAD_SLOT, np.float32)
        off = 0
        for b in range(NBLK):
            i, sl = blocks[b]
            n = len(i)
            idx[off:off + n] = i
            slot[off:off + n] = sl
            off += int(T[b]) * P
        return idx, slot

    in_maps = []
    deg_out = np.bincount(src, minlength=N_NODES).astype(np.float32)
    r_out_full = 1.0 / np.sqrt(np.clip(deg_out, 1.0, None))
    deg_in = np.bincount(dst, minlength=N_NODES).astype(np.float32)
    r_in_full = 1.0 / np.sqrt(np.clip(deg_in, 1.0, None))

    featT = np.zeros((INF_PAD, N_NODES), np.float32)
    featT[:IN_F] = features.T
    W1p = np.zeros((INF_PAD, HID), np.float32)
    W1p[:IN_F] = W1

    for c in range(NCORES):
        blocksA, blocksB = per_core[c]
        idxA, slotA = build_half(blocksA, TA, NA)
        idxB, slotB = build_half(blocksB, TB, NB)
        slotAB = np.concatenate([slotA, slotB]).reshape(TA_tot + TB_tot, P).T

        # r_out for this shard, node-partition-wrapped [128, NBLK]
        ro = np.zeros(NBLK * P, np.float32)
        ro[:SH] = r_out_full[c * SH:(c + 1) * SH]
        r_out_c = ro.reshape(NBLK, P).T.copy()

        # r_in for this shard, pre-broadcast to [128, SH]
        r_in_c = np.tile(r_in_full[c * SH:(c + 1) * SH][None, :], (P, 1))

        # featT shard as [P, KCH, SH] so one 3D-AP DMA loads a [128,KCH,slab]
        featTc = featT[:, c * SH:(c + 1) * SH].reshape(KCH, P, SH)
        in_maps.append({
            "featT": featTc.transpose(1, 0, 2).astype(BF16).copy(),
            "W1c": W1p.reshape(KCH, P, HID).transpose(1, 0, 2).astype(BF16).copy(),
            "W2c": W2.astype(BF16), "W3c": W3.astype(BF16),
            "Wfcc": Wfc.astype(BF16),
            "b1c": b1.reshape(HID, 1).astype(np.float32),
            "b2c": b2.reshape(HID, 1).astype(np.float32),
            "b3c": b3.reshape(HID, 1).astype(np.float32),
            "bfcc": bfc.reshape(1, OUT_F).astype(BF16),
            "iota": np.tile(np.arange(P, dtype=np.float32).astype(BF16), (P, 1)),
            "ones_b": np.ones((1, P), BF16),
            "idxA": _wrap_idx16(idxA), "idxB": _wrap_idx16(idxB),
            "slotAB": slotAB.astype(BF16),
            "r_out": r_out_c,
            "r_in_bc": r_in_c.astype(np.float32),
        })

    sched = {"TA": TA.tolist(), "TB": TB.tolist(),
             "TA_tot": TA_tot, "TB_tot": TB_tot}
    return in_maps, sched


# ------------------------------------------------------------- device program

def _build(sched):
    TA, TB = sched["TA"], sched["TB"]
    TA_tot, TB_tot = sched["TA_tot"], sched["TB_tot"]
    EA, EB = TA_tot * P, TB_tot * P

    nc = bacc.Bacc("TRN2", target_bir_lowering=False, debug=False,
                   num_devices=NCORES, num_swdge_queues=4)

    def din(name, shape, dt):
        return nc.dram_tensor(name, shape, dt, kind="ExternalInput")

    h = {
        "featT": din("featT", [P, KCH, SH], BF),
        "W1c": din("W1c", [P, KCH, HID], BF),
        "W2c": din("W2c", [HID, HID], BF),
        "W3c": din("W3c", [HID, HID], BF),
        "Wfcc": din("Wfcc", [HID, OUT_F], BF),
        "b1c": din("b1c", [HID, 1], F32),
        "b2c": din("b2c", [HID, 1], F32),
        "b3c": din("b3c", [HID, 1], F32),
        "bfcc": din("bfcc", [1, OUT_F], BF),
        "iota": din("iota", [P, P], BF),
        "ones_b": din("ones_b", [1, P], BF),
        "idxA": din("idxA", [P, EA // 16], I16),
        "idxB": din("idxB", [P, EB // 16], I16),
        "slotAB": din("slotAB", [P, TA_tot + TB_tot], BF),
        "r_out": din("r_out", [P, NBLK], F32),
        "r_in_bc": din("r_in_bc", [P, SH], F32),
    }
    out_fc = nc.dram_tensor("out_fc", [SH, OUT_F], F32, kind="ExternalOutput")
    Tshard = [nc.dram_tensor(f"Tshard{i}", [SH, HID], BF) for i in range(3)]
    TfullA = [nc.dram_tensor(f"TfullA{i}", [NA, HID], BF, addr_space="Shared")
              for i in range(3)]
    TfullB = [nc.dram_tensor(f"TfullB{i}", [NB, HID], BF, addr_space="Shared")
              for i in range(3)]

    with tile.TileContext(nc) as tc, \
         tc.tile_pool(name="const", bufs=1) as cp, \
         tc.tile_pool(name="state", bufs=1) as statep, \
         tc.tile_pool(name="feat", bufs=2) as featp, \
         tc.tile_pool(name="msgA", bufs=10) as msgAp, \
         tc.tile_pool(name="msgB", bufs=10) as msgBp, \
         tc.tile_pool(name="sTA", bufs=4) as sTAp, \
         tc.tile_pool(name="sTB", bufs=4) as sTBp, \
         tc.tile_pool(name="stage", bufs=8) as stagep, \
         tc.tile_pool(name="ps_agg", bufs=4, space="PSUM") as ps_agg, \
         tc.tile_pool(name="ps_dense", bufs=3, space="PSUM") as ps_dense, \
         tc.tile_pool(name="ps_fc", bufs=1, space="PSUM") as ps_fc:

        def load(name, shape, dt):
            t = cp.tile(shape, dt, tag=name)
            nc.sync.dma_start(out=t[:], in_=h[name][:])
            return t

        W1s = load("W1c", [P, KCH, HID], BF)
        W2s = load("W2c", [HID, HID], BF)
        W3s = load("W3c", [HID, HID], BF)
        Wfcs = load("Wfcc", [HID, OUT_F], BF)
        b1s = load("b1c", [HID, 1], F32)
        b2s = load("b2c", [HID, 1], F32)
        b3s = load("b3c", [HID, 1], F32)
        bfcs = load("bfcc", [1, OUT_F], BF)
        iota = load("iota", [P, P], BF)
        ones_b = load("ones_b", [1, P], BF)
        idxA = load("idxA", [P, EA // 16], I16)
        idxB = load("idxB", [P, EB // 16], I16)
        slotAB = load("slotAB", [P, TA_tot + TB_tot], BF)
        r_out = load("r_out", [P, NBLK], F32)
        r_in_b = statep.tile([P, SH], F32, tag="r_in_b")
        nc.sync.dma_start(out=r_in_b[:], in_=h["r_in_bc"][:])

        x1 = statep.tile([P, SH], BF, tag="x1")
        x2 = statep.tile([P, SH], BF, tag="x2")

        def dense_block(li, xin, Wt, nb, m):
            """Tshard[li][block nb] = r_out * (xin[:, block nb] @ Wt)."""
            ps = ps_dense.tile([P, HID], F32, space="PSUM")
            nc.tensor.matmul(ps[:m, :], lhsT=xin[:, nb * P:nb * P + m],
                             rhs=Wt[:], start=True, stop=True)
            st = stagep.tile([P, HID], BF, tag="st")
            nc.vector.tensor_scalar_mul(out=st[:m, :], in0=ps[:m, :],
                                        scalar1=r_out[:m, nb:nb + 1])
            nc.sync.dma_start(out=Tshard[li][nb * P:nb * P + m, :], in_=st[:m, :])

        def all_gather(li, half):
            if half == "A":
                ins, outs = Tshard[li][0:ASH, :], TfullA[li][:]
            else:
                ins, outs = Tshard[li][ASH:SH, :], TfullB[li][:]
            nc.gpsimd.collective_compute(
                "AllGather", mybir.AluOpType.bypass,
                replica_groups=[list(range(NCORES))],
                ins=[ins.opt()], outs=[outs.opt()])

        # ---- T1 = r_out * (features @ W1), feature slabs streamed from DRAM
        for j0 in range(0, SH, SLABW):
            w = min(SLABW, SH - j0)
            ft = featp.tile([P, KCH, SLABW], BF, tag="ft")
            nc.sync.dma_start(out=ft[:, :, :w], in_=h["featT"][:, :, j0:j0 + w])
            for nb in range(j0 // P, (j0 + w + P - 1) // P):
                c0 = nb * P - j0
                m = min(P, w - c0)
                ps = ps_dense.tile([P, HID], F32, space="PSUM")
                for k in range(KCH):
                    nc.tensor.matmul(ps[:m, :],
                                     lhsT=ft[:, k, c0:c0 + m],
                                     rhs=W1s[:, k, :],
                                     start=(k == 0), stop=(k == KCH - 1))
                st = stagep.tile([P, HID], BF, tag="st")
                nc.vector.tensor_scalar_mul(out=st[:m, :], in0=ps[:m, :],
                                            scalar1=r_out[:m, nb:nb + 1])
                nc.sync.dma_start(out=Tshard[0][nb * P:nb * P + m, :], in_=st[:m, :])
                if nb == ABLK - 1:
                    all_gather(0, "A")
        all_gather(0, "B")

        # ---- aggregation machinery
        def emit_gather(li, tot_tiles, idx_t, base_ap, pool, k, q):
            nt = min(CHUNK_TILES, tot_tiles - k * CHUNK_TILES)
            mt = pool.tile([P, nt, HID], BF, tag="msg")
            nidx = nt * P
            c0 = k * CHUNK_TILES * P // 16
            nc.gpsimd.dma_gather(
                out_ap=mt[:], in_ap=base_ap, idxs_ap=idx_t[:, c0:c0 + nidx // 16],
                num_idxs=nidx, num_idxs_reg=nidx, elem_size=HID,
                queue_num=q % 4)
            return mt

        def gathers_interleaved(li):
            """Emit A/B gather calls alternating (matches per-block A-then-B
            consumption order) round-robin across the 4 SWDGE queues."""
            ncA = (TA_tot + CHUNK_TILES - 1) // CHUNK_TILES
            ncB = (TB_tot + CHUNK_TILES - 1) // CHUNK_TILES
            mA, mB = [], []
            q = 0
            for k in range(max(ncA, ncB)):
                if k < ncA:
                    mA.append(emit_gather(li, TA_tot, idxA,
                                          TfullA[li][:], msgAp, k, q))
                    q += 1
                if k < ncB:
                    mB.append(emit_gather(li, TB_tot, idxB,
                                          TfullB[li][:], msgBp, k, q))
                    q += 1
            return mA, mB

        def st_group(pool, slot_off, g, tot_tiles):
            nt = min(ST_GROUP, tot_tiles - g * ST_GROUP)
            t = pool.tile([P, ST_GROUP, P], BF, tag="sT")
            sl = slotAB[:, slot_off + g * ST_GROUP: slot_off + g * ST_GROUP + nt]
            nc.vector.tensor_tensor(
                out=t[:, :nt, :],
                in0=sl.unsqueeze(2).to_broadcast([P, nt, P]),
                in1=iota[:].unsqueeze(1).to_broadcast([P, nt, P]),
                op=mybir.AluOpType.is_equal)
            return t

        def aggregate(li, bias, xout, dense_W=None, fc=False):
            """Per dst block: one-hot matmul segment-sum, then fused tail:
            xout[:,blk] = relu(r_in * agg + bias); plus either the NEXT
            layer's dense+scale+store for this block (dense_W) or the final
            residual+FC+store (fc).  The next layer's A-half AllGather fires
            right after block 24's dense store."""
            mA, mB = gathers_interleaved(li)
            sA, sB = {}, {}
            tA = tB = 0
            for b in range(NBLK):
                ps = ps_agg.tile([P, P], F32, space="PSUM")
                tot = TA[b] + TB[b]
                i = 0
                for (cnt, cur, msgs, sTs, pool, soff, ttot) in (
                        (TA[b], tA, mA, sA, sTAp, 0, TA_tot),
                        (TB[b], tB, mB, sB, sTBp, TA_tot, TB_tot)):
                    for t in range(cur, cur + cnt):
                        g = t // ST_GROUP
                        if g not in sTs:
                            sTs[g] = st_group(pool, soff, g, ttot)
                        nc.tensor.matmul(
                            ps[:],
                            lhsT=msgs[t // CHUNK_TILES][:, t % CHUNK_TILES, :],
                            rhs=sTs[g][:, t % ST_GROUP, :],
                            start=(i == 0), stop=(i == tot - 1))
                        i += 1
                tA += TA[b]
                tB += TB[b]
                m = P if b < NBLK - 1 else LASTM
                bsl = slice(b * P, b * P + m)
                mulst = stagep.tile([P, P], BF, tag="mulst")
                nc.vector.tensor_mul(out=mulst[:, :m], in0=ps[:, :m],
                                     in1=r_in_b[:, bsl])
                nc.scalar.activation(xout[:, bsl], mulst[:, :m],
                                     mybir.ActivationFunctionType.Relu,
                                     bias=bias[:])
                if dense_W is not None:
                    dense_block(li + 1, xout, dense_W, b, m)
                    if b == ABLK - 1:
                        all_gather(li + 1, "A")
                if fc:
                    x4b = stagep.tile([P, P], BF, tag="x4b")
                    nc.vector.tensor_add(out=x4b[:, :m], in0=xout[:, bsl],
                                         in1=x2[:, bsl])
                    psf = ps_fc.tile([P, OUT_F], F32, space="PSUM")
                    nc.tensor.matmul(psf[:m, :], lhsT=x4b[:, :m],
                                     rhs=Wfcs[:], start=True, stop=False)
                    nc.tensor.matmul(psf[:m, :], lhsT=ones_b[:, :m], rhs=bfcs[:],
                                     start=False, stop=True)
                    stf = stagep.tile([P, OUT_F], F32, tag="stf")
                    nc.vector.tensor_copy(out=stf[:m, :], in_=psf[:m, :])
                    nc.sync.dma_start(out=out_fc[b * P:b * P + m, :],
                                      in_=stf[:m, :])
            if dense_W is not None:
                all_gather(li + 1, "B")

        x3 = statep.tile([P, SH], BF, tag="x1")   # reuse x1 slot (x1 dead then)
        aggregate(0, b1s, x1, dense_W=W2s)   # x1 = relu(gconv(feat,W1)); T2; AG
        aggregate(1, b2s, x2, dense_W=W3s)   # x2 = relu(gconv(x1,W2));  T3; AG
        aggregate(2, b3s, x3, fc=True)       # x3 + residual + FC + store

    nc.compile()
    return nc


_CACHED = None


def kernel(**inputs):
    global _CACHED
    in_maps, sched = _preprocess(**inputs)
    if _CACHED is None or _CACHED[1] != sched:
        _CACHED = (_build(sched), sched)
    nc = _CACHED[0]
    res = run_bass_kernel_spmd(nc, in_maps, list(range(NCORES)))
    return np.concatenate(
        [np.asarray(res.results[c]["out_fc"], np.float32) for c in range(NCORES)], 0)
